# revision 31
# baseline (speedup 1.0000x reference)
"""Trainium2 Bass kernel for nn_CausalDiscoveryLayer (27-node Granger + MHA).

Contract: kernel(**inputs) takes FULL unsharded numpy inputs and returns the
full output (out [27,1024,5], causal_adj [27,27]) matching the reference.

Strategy (8 NeuronCores, tensor-parallel on the two big weight matrices):
  Launch 1: in_proj (15360x5120, 315MB) sharded head-aligned 1920 rows/core
            (core c = head c//2, hd-half c%2: its 640-row q, k and v slices);
            each core computes its qkv^T slice [15x128, 27], partial
            attention scores for its head, and the (replicated) Granger
            adjacency.
  Host:     sums the pair-partial scores, carves v^T slabs.
  Launch 2: out_proj (5120x5120, 105MB) sharded over the contraction dim
            (640/core = the core's hd-half); softmax + attention + partial
            output out^T [5120, 27]; host sums the 8 partials.

Weights are transposed/relaid on the host so every DMA is one contiguous
run per SBUF partition (fp32 has no HW DMA-transpose path).

The big matmuls use a bf16 high/low split (x = xh + xl, W = Wh + Wl;
x@W ~= xh@Wh + xl@Wh + xh@Wl): same HBM bytes as fp32, ~5e-6 relative
error (fp32-grade). Weights ride the PE stationary port (bf16 fast-weight
-load), activations are the 27-wide moving operand. Granger/softmax math
stays plain fp32.
"""

import numpy as np
import ml_dtypes

import os

import concourse.bass as bass
import concourse.bacc as bacc
import concourse.mybir as mybir
import concourse.tile as tile
from concourse.bass_utils import run_bass_kernel_spmd
from concourse.masks import make_identity

N = 27
D = 1024
E = 5120
H = 4
HD = E // H          # 1280
NCORES = 8
P = 128
KC = E // P          # 40 contraction chunks of 128
JC = 3 * E // NCORES  # 1920 in_proj rows per core (640 each of q/k/v)
JB = JC // P         # 15 output blocks of 128
FS = E // NCORES     # 640 out_proj contraction rows per core
FB = FS // P         # 5 f blocks
EB = E // P          # 40 out_proj output blocks
FP32 = mybir.dt.float32
BF16 = mybir.dt.bfloat16
NPBF = ml_dtypes.bfloat16

# Results of the last run (BassKernelResults per launch) for test harnesses.
LAST_RESULTS = []

_NC_CACHE = {}


def _build_launch1():
    nc = bacc.Bacc("TRN2", target_bir_lowering=False, debug=False,
                   num_devices=NCORES)
    # wt: per-output-block slabs; one contiguous 20.5KB run per partition.
    wt = nc.dram_tensor("wt", [JB, P, KC, 2, P], BF16, kind="ExternalInput")
    biasT = nc.dram_tensor("biasT", [1, JB, P], FP32, kind="ExternalInput")
    xT = nc.dram_tensor("xT", [P, KC, N], FP32, kind="ExternalInput")
    xb = nc.dram_tensor("xb", [P, KC, 2, N], BF16, kind="ExternalInput")
    histT = nc.dram_tensor("histT", [P, KC, N], FP32, kind="ExternalInput")
    wn = nc.dram_tensor("wn", [P, KC, 1], FP32, kind="ExternalInput")
    wh = nc.dram_tensor("wh", [P, KC, 1], FP32, kind="ExternalInput")
    gb = nc.dram_tensor("gb", [1, 1], FP32, kind="ExternalInput")
    mask = nc.dram_tensor("mask", [N, N], FP32, kind="ExternalInput")
    qkvT = nc.dram_tensor("qkvT", [P, JB, N], FP32, kind="ExternalOutput")
    scp = nc.dram_tensor("scp", [N, N], FP32, kind="ExternalOutput")
    adj = nc.dram_tensor("adj", [N, N], FP32, kind="ExternalOutput")

    with tile.TileContext(nc) as tc:
        with (
            tc.tile_pool(name="const", bufs=1) as const,
            tc.tile_pool(name="slab", bufs=3) as slabp,
            tc.tile_pool(name="acc", bufs=3, space="PSUM") as accp,
            tc.tile_pool(name="gps", bufs=1, space="PSUM") as gpsp,
        ):
            ones_f32 = const.tile([1, N], FP32)
            nc.gpsimd.memset(ones_f32[:], 1.0)
            xT_sb = const.tile([P, KC, N], FP32)
            nc.scalar.dma_start(xT_sb[:], xT[:])
            xb_sb = const.tile([P, KC, 2, N], BF16)
            nc.scalar.dma_start(xb_sb[:], xb[:])
            histT_sb = const.tile([P, KC, N], FP32)
            nc.scalar.dma_start(histT_sb[:], histT[:])
            wn_sb = const.tile([P, KC, 1], FP32)
            nc.scalar.dma_start(wn_sb[:], wn[:])
            wh_sb = const.tile([P, KC, 1], FP32)
            nc.scalar.dma_start(wh_sb[:], wh[:])
            gb_sb = const.tile([1, 1], FP32)
            nc.scalar.dma_start(gb_sb[:], gb[:])
            mask_sb = const.tile([N, N], FP32)
            nc.scalar.dma_start(mask_sb[:], mask[:])
            biasT_sb = const.tile([1, JB, P], FP32)
            nc.scalar.dma_start(biasT_sb[:], biasT[:])

            # qkvT[jb*128+j, n] = sum_k W[row jb*128+j, k] x[n, k] + b[row]
            qkvT_sb = const.tile([P, JB, N], FP32)
            for jb in range(JB):
                slab = slabp.tile([P, KC, 2, P], BF16, tag="slab", name="slab")
                nc.sync.dma_start(slab[:], wt[jb])
                ps = accp.tile([P, N], FP32, tag="acc", name="accT")
                for kc in range(KC):
                    nc.tensor.matmul(ps[:], slab[:, kc, 0, :], xb_sb[:, kc, 0, :],
                                     start=(kc == 0), stop=False)
                    nc.tensor.matmul(ps[:], slab[:, kc, 0, :], xb_sb[:, kc, 1, :],
                                     start=False, stop=False)
                    nc.tensor.matmul(ps[:], slab[:, kc, 1, :], xb_sb[:, kc, 0, :],
                                     start=False, stop=False)
                # bias: biasT_chunk^T [P,1] @ ones [1,N]
                nc.tensor.matmul(ps[:], biasT_sb[:, jb, :], ones_f32[:],
                                 start=False, stop=True)
                nc.vector.tensor_copy(qkvT_sb[:, jb, :], ps[:])

            # partial scores for this core's hd-half of its head:
            # scp[q, t] = sum_{d in half} qT[d, q] kT[d, t] / sqrt(hd)
            sc_ps = gpsp.tile([N, N], FP32, tag="scps")
            for b in range(FB):
                nc.tensor.matmul(sc_ps[:], qkvT_sb[:, b, :], qkvT_sb[:, FB + b, :],
                                 start=(b == 0), stop=(b == FB - 1))
            sc_sb = const.tile([N, N], FP32)
            nc.scalar.mul(sc_sb[:], sc_ps[:], 1.0 / float(np.sqrt(np.float32(HD))))
            nc.scalar.dma_start(scp[:], sc_sb[:])
            nc.scalar.dma_start(qkvT[:], qkvT_sb[:])

            # Granger: col[i] = x[i,:].wn + gb ; row[j] = hist[j,:].wh
            col_ps = gpsp.tile([N, 1], FP32, tag="col")
            for kc in range(KC):
                nc.tensor.matmul(col_ps[:], xT_sb[:, kc, :], wn_sb[:, kc, :],
                                 start=(kc == 0), stop=False)
            nc.tensor.matmul(col_ps[:], ones_f32[:], gb_sb[:], start=False, stop=True)
            row_ps = gpsp.tile([1, N], FP32, tag="row")
            for kc in range(KC):
                nc.tensor.matmul(row_ps[:], wh_sb[:, kc, :], histT_sb[:, kc, :],
                                 start=(kc == 0), stop=(kc == KC - 1))
            col_sb = const.tile([N, 1], FP32)
            nc.vector.tensor_copy(col_sb[:], col_ps[:])
            row_sb = const.tile([1, N], FP32)
            nc.vector.tensor_copy(row_sb[:], row_ps[:])
            rowb_ps = gpsp.tile([N, N], FP32, tag="rowb")
            nc.tensor.matmul(rowb_ps[:], ones_f32[:], row_sb[:], start=True, stop=True)
            adj_sb = const.tile([N, N], FP32)
            nc.scalar.activation(adj_sb[:], rowb_ps[:],
                                 mybir.ActivationFunctionType.Sigmoid,
                                 bias=col_sb[:])
            nc.vector.tensor_mul(adj_sb[:], adj_sb[:], mask_sb[:])
            nc.scalar.dma_start(adj[:], adj_sb[:])
    nc.compile()
    return nc


def _build_launch2():
    nc = bacc.Bacc("TRN2", target_bir_lowering=False, debug=False,
                   num_devices=NCORES)
    EG = 4               # 128-blocks of e per weight slab
    NEG = EB // EG       # 10 slabs
    wt2 = nc.dram_tensor("wt2", [NEG, P, FB, 2, EG * P], BF16,
                         kind="ExternalInput")
    sc = nc.dram_tensor("sc", [N, N], FP32, kind="ExternalInput")
    vT = nc.dram_tensor("vT", [P, FB, N], FP32, kind="ExternalInput")
    outT = nc.dram_tensor("outT", [P, EB, N], FP32, kind="ExternalOutput")

    with tile.TileContext(nc) as tc:
        with (
            tc.tile_pool(name="const", bufs=1) as const,
            tc.tile_pool(name="w2", bufs=6) as w2p,
            tc.tile_pool(name="att_ps", bufs=1, space="PSUM") as attps,
            tc.tile_pool(name="ot_ps", bufs=2, space="PSUM") as otps,
            tc.tile_pool(name="out_ps", bufs=3, space="PSUM") as outps,
        ):
            # Preload the Exp activation table while DMAs run.
            warm = const.tile([1, 1], FP32)
            nc.gpsimd.memset(warm[:], 0.0)
            nc.scalar.activation(warm[:], warm[:],
                                 mybir.ActivationFunctionType.Exp)

            sc_sb = const.tile([N, N], FP32)
            nc.scalar.dma_start(sc_sb[:], sc[:])
            vT_sb = const.tile([P, FB, N], FP32)
            nc.scalar.dma_start(vT_sb[:], vT[:])

            w2_sb = []
            for eg in range(NEG):
                t = w2p.tile([P, FB, 2, EG * P], BF16, tag="w2", name="w2sb")
                nc.sync.dma_start(t[:], wt2[eg])
                w2_sb.append(t)

            ident = const.tile([N, N], FP32)
            make_identity(nc, ident[:])
            identP = const.tile([P, P], FP32)
            make_identity(nc, identP[:])

            # softmax over pair-summed scores
            nmax = const.tile([N, 1], FP32)
            nc.vector.reduce_max(nmax[:], sc_sb[:], axis=mybir.AxisListType.X)
            nc.scalar.mul(nmax[:], nmax[:], -1.0)
            exp_sb = const.tile([N, N], FP32)
            nc.scalar.activation(exp_sb[:], sc_sb[:],
                                 mybir.ActivationFunctionType.Exp, bias=nmax[:])
            ssum = const.tile([N, 1], FP32)
            nc.vector.reduce_sum(ssum[:], exp_sb[:], axis=mybir.AxisListType.X)
            rec = const.tile([N, 1], FP32)
            nc.vector.reciprocal(rec[:], ssum[:])
            attn_sb = const.tile([N, N], FP32)
            nc.vector.tensor_scalar_mul(attn_sb[:], exp_sb[:], rec[:])

            # attn^T via PE transpose
            at_ps = attps.tile([N, N], FP32, tag="at")
            nc.tensor.transpose(at_ps[:], attn_sb[:], ident[:])
            attnT_sb = const.tile([N, N], FP32)
            nc.vector.tensor_copy(attnT_sb[:], at_ps[:])

            # v natural blocks [27, 128] from vT via PE transpose, then
            # o^T[f, q] = sum_t v[t, f] attn^T[t, q], split into bf16 hi/lo
            v_sb = const.tile([N, FB, P], FP32)
            for b in range(FB):
                vt_ps = otps.tile([N, P], FP32, tag="vt", name="vtps")
                nc.tensor.transpose(vt_ps[:], vT_sb[:, b, :], identP[:])
                nc.vector.tensor_copy(v_sb[:, b, :], vt_ps[:])

            oT_sb = const.tile([P, FB, N], FP32)
            ohl_sb = const.tile([P, FB, 2, N], BF16)
            oh32 = const.tile([P, FB, N], FP32)
            for b in range(FB):
                o_ps = otps.tile([P, N], FP32, tag="ot", name="otps")
                nc.tensor.matmul(o_ps[:], v_sb[:, b, :], attnT_sb[:],
                                 start=True, stop=True)
                nc.vector.tensor_copy(oT_sb[:, b, :], o_ps[:])
                nc.vector.tensor_copy(ohl_sb[:, b, 0, :], o_ps[:])
                nc.vector.tensor_copy(oh32[:, b, :], ohl_sb[:, b, 0, :])
                nc.vector.tensor_sub(oT_sb[:, b, :], oT_sb[:, b, :], oh32[:, b, :])
                nc.vector.tensor_copy(ohl_sb[:, b, 1, :], oT_sb[:, b, :])

            # outT[e, n] = sum_f w2[f, e] oT[f, n]  (partial over f slice)
            outT_sb = const.tile([P, EB, N], FP32)
            for eg in range(NEG):
                for eb in range(EG):
                    esl = slice(eb * P, (eb + 1) * P)
                    op_ps = outps.tile([P, N], FP32, tag="out", name="outps")
                    for fc in range(FB):
                        last = fc == FB - 1
                        nc.tensor.matmul(op_ps[:], w2_sb[eg][:, fc, 0, esl],
                                         ohl_sb[:, fc, 0, :],
                                         start=(fc == 0), stop=False)
                        nc.tensor.matmul(op_ps[:], w2_sb[eg][:, fc, 0, esl],
                                         ohl_sb[:, fc, 1, :],
                                         start=False, stop=False)
                        nc.tensor.matmul(op_ps[:], w2_sb[eg][:, fc, 1, esl],
                                         ohl_sb[:, fc, 0, :],
                                         start=False, stop=last)
                    e = eg * EG + eb
                    nc.vector.tensor_copy(outT_sb[:, e, :], op_ps[:])
            nc.scalar.dma_start(outT[:], outT_sb[:])
    nc.compile()
    return nc


def _chunked_T(a2d):
    """[R, C] -> transposed chunk layout [128, R//128, C] (k on partitions)."""
    r, c = a2d.shape
    return np.ascontiguousarray(
        a2d.reshape(r // P, P, c).transpose(1, 0, 2)).astype(np.float32)


def _split_hl(a, axis):
    """Stack bf16 high/low parts of a float32 array along a new axis."""
    ah = a.astype(NPBF)
    al = (a - ah.astype(np.float32)).astype(NPBF)
    return np.ascontiguousarray(np.stack([ah, al], axis=axis))


def _w_slabs(w_rows, nblk, kcnt, blk):
    """[Rows, K] fp32 -> [nblk, 128(k-part), kcnt, 2(hi/lo), blk(row)] bf16.

    Each output-row block becomes one slab whose per-partition bytes are one
    contiguous DRAM run.
    """
    hl = _split_hl(w_rows.T, axis=1)           # [K, 2, Rows] bf16
    arr = hl.reshape(kcnt, P, 2, nblk, blk)    # [kc, p, t, blk, j]
    return np.ascontiguousarray(arr.transpose(3, 1, 0, 2, 4))


def _run_spmd(nc, in_maps):
    try:
        return run_bass_kernel_spmd(nc, in_maps, list(range(NCORES)))
    except Exception:
        # Trace/profile plumbing (BASS_TRACE) can fail in minimal
        # environments; the untraced path only needs PJRT.
        os.environ["BASS_NEVER_TRACE"] = "1"
        return run_bass_kernel_spmd(nc, in_maps, list(range(NCORES)))


def kernel(node_states, hist_prev, granger_w, granger_b,
           in_proj_w, in_proj_b, out_proj_w, out_proj_b, light_hertz):
    global LAST_RESULTS
    LAST_RESULTS = []
    node_states = np.asarray(node_states, dtype=np.float32)
    hist_prev = np.asarray(hist_prev, dtype=np.float32)
    granger_w = np.asarray(granger_w, dtype=np.float32)
    granger_b = np.asarray(granger_b, dtype=np.float32)
    in_proj_w = np.asarray(in_proj_w, dtype=np.float32)
    in_proj_b = np.asarray(in_proj_b, dtype=np.float32)
    out_proj_w = np.asarray(out_proj_w, dtype=np.float32)
    out_proj_b = np.asarray(out_proj_b, dtype=np.float32)

    x = node_states.reshape(N, E)
    hist = hist_prev.reshape(N, E)

    xT = _chunked_T(x.T.reshape(E, N))          # [128, 40, 27]
    xb = _split_hl(xT, axis=2)                  # [128, 40, 2, 27] bf16
    histT = _chunked_T(hist.T.reshape(E, N))
    wh = _chunked_T(granger_w[0, :E].reshape(E, 1))
    wn = _chunked_T(granger_w[0, E:].reshape(E, 1))
    gb = granger_b.reshape(1, 1)
    mask = (1.0 - np.eye(N, dtype=np.float32))

    if "l1" not in _NC_CACHE:
        _NC_CACHE["l1"] = _build_launch1()
    if "l2" not in _NC_CACHE:
        _NC_CACHE["l2"] = _build_launch2()

    in_maps1 = []
    row_slices = []
    for c in range(NCORES):
        h, half = divmod(c, 2)
        base = h * HD + half * FS
        rows = np.r_[base:base + FS, E + base:E + base + FS,
                     2 * E + base:2 * E + base + FS]
        row_slices.append(rows)
        in_maps1.append({
            "wt": _w_slabs(in_proj_w[rows, :], JB, KC, P),
            "biasT": np.ascontiguousarray(
                in_proj_b[rows].reshape(1, JB, P)),
            "xT": xT, "xb": xb, "histT": histT, "wn": wn, "wh": wh, "gb": gb,
            "mask": mask,
        })
    res1 = _run_spmd(_NC_CACHE["l1"], in_maps1)
    LAST_RESULTS.append(res1)

    causal_adj = np.asarray(res1.results[0]["adj"])
    hw = float(np.clip(np.float32(light_hertz) / np.float32(1000.0), 0.1, 1.0))

    in_maps2 = []
    for c in range(NCORES):
        h, half = divmod(c, 2)
        fsl = slice(h * HD + half * FS, h * HD + half * FS + FS)
        # pair-summed scores for this head
        scores = res1.results[2 * h]["scp"] + res1.results[2 * h + 1]["scp"]
        # own v^T slab: blocks 10..14 of qkvT, hertz-scaled
        vt = res1.results[c]["qkvT"][:, 2 * FB:3 * FB, :] * np.float32(hw)
        in_maps2.append({
            "wt2": _w_slabs(np.ascontiguousarray(out_proj_w[:, fsl]), EB // 4, FB, 4 * P),
            "sc": scores,
            "vT": np.ascontiguousarray(vt),  # [128, 5, 27]
        })
    res2 = _run_spmd(_NC_CACHE["l2"], in_maps2)
    LAST_RESULTS.append(res2)

    outT = np.zeros((E, N), dtype=np.float32)
    for c in range(NCORES):
        outT += res2.results[c]["outT"].transpose(1, 0, 2).reshape(E, N)
    out = outT.T + np.float32(hw) * out_proj_b
    return np.ascontiguousarray(out).reshape(N, D, 5), causal_adj


# revision 32
# speedup vs baseline: 1.0989x; 1.0989x over previous
"""Trainium2 Bass kernel for nn_CausalDiscoveryLayer (27-node Granger + MHA).

Contract: kernel(**inputs) takes FULL unsharded numpy inputs and returns the
full output (out [27,1024,5], causal_adj [27,27]) matching the reference.

Strategy (8 NeuronCores, tensor-parallel on the two big weight matrices):
  Launch 1: in_proj (15360x5120, 315MB) sharded 1920 rows/core; each core
            computes its qkv slice [27,1920]; Granger adjacency replicated.
  Host:     reassemble qkv, carve per-head q^T,k^T and v slices.
  Launch 2: out_proj (5120x5120, 105MB) sharded over the contraction dim
            (640/core = half a head); each core runs softmax-attention for
            its head and produces a partial output [27,5120]; host sums.

Weights are transposed on the host so that the contraction dimension lies on
SBUF partitions with unit-stride DMA (fp32 has no HW DMA-transpose path).

The big matmuls use a bf16 high/low split (x = xh + xl, W = Wh + Wl;
x@W ~= xh@Wh + xl@Wh + xh@Wl): same HBM bytes as fp32, 3 PE cycles/row
instead of fp32's 4, and ~5e-6 relative error (fp32-grade). The tiny
Granger/attention math stays plain fp32.
"""

import numpy as np
import ml_dtypes

import concourse.bass as bass
import concourse.bacc as bacc
import concourse.mybir as mybir
import concourse.tile as tile
from concourse.bass_utils import run_bass_kernel_spmd
from concourse.masks import make_identity

N = 27
D = 1024
E = 5120
H = 4
HD = E // H          # 1280
NCORES = 8
P = 128
KC = E // P          # 40 contraction chunks of 128
JC = 3 * E // NCORES  # 1920 in_proj output cols per core
JT = 480             # matmul free-dim tile for launch 1 (4 * 480 = 1920)
FS = E // NCORES     # 640 out_proj contraction rows per core
FP32 = mybir.dt.float32
BF16 = mybir.dt.bfloat16
NPBF = ml_dtypes.bfloat16

# Results of the last run (BassKernelResults per launch) for test harnesses.
LAST_RESULTS = []

_NC_CACHE = {}


def _build_launch1():
    nc = bacc.Bacc("TRN2", target_bir_lowering=False, debug=False,
                   num_devices=NCORES)
    wt = nc.dram_tensor("wt", [E, 2, JC], BF16, kind="ExternalInput")
    xT = nc.dram_tensor("xT", [P, KC, N], FP32, kind="ExternalInput")
    xb = nc.dram_tensor("xb", [P, KC, 2, N], BF16, kind="ExternalInput")
    histT = nc.dram_tensor("histT", [P, KC, N], FP32, kind="ExternalInput")
    wn = nc.dram_tensor("wn", [P, KC, 1], FP32, kind="ExternalInput")
    wh = nc.dram_tensor("wh", [P, KC, 1], FP32, kind="ExternalInput")
    gb = nc.dram_tensor("gb", [1, 1], FP32, kind="ExternalInput")
    mask = nc.dram_tensor("mask", [N, N], FP32, kind="ExternalInput")
    qkv = nc.dram_tensor("qkv", [N, JC], FP32, kind="ExternalOutput")
    adj = nc.dram_tensor("adj", [N, N], FP32, kind="ExternalOutput")

    with tile.TileContext(nc) as tc:
        with (
            tc.tile_pool(name="const", bufs=1) as const,
            tc.tile_pool(name="rhs", bufs=6) as rhsp,
            tc.tile_pool(name="outsb", bufs=1) as outsb,
            tc.tile_pool(name="acc", bufs=4, space="PSUM") as accp,
            tc.tile_pool(name="gps", bufs=1, space="PSUM") as gpsp,
        ):
            ones_f32 = const.tile([1, N], FP32)
            nc.gpsimd.memset(ones_f32[:], 1.0)
            xT_sb = const.tile([P, KC, N], FP32)
            nc.sync.dma_start(xT_sb[:], xT[:])
            xb_sb = const.tile([P, KC, 2, N], BF16)
            nc.sync.dma_start(xb_sb[:], xb[:])
            histT_sb = const.tile([P, KC, N], FP32)
            nc.sync.dma_start(histT_sb[:], histT[:])
            wn_sb = const.tile([P, KC, 1], FP32)
            nc.sync.dma_start(wn_sb[:], wn[:])
            wh_sb = const.tile([P, KC, 1], FP32)
            nc.sync.dma_start(wh_sb[:], wh[:])
            gb_sb = const.tile([1, 1], FP32)
            nc.sync.dma_start(gb_sb[:], gb[:])
            mask_sb = const.tile([N, N], FP32)
            nc.sync.dma_start(mask_sb[:], mask[:])

            out_sb = outsb.tile([N, JC], FP32)

            # qkv_slice[n, j] = sum_k x[n,k] * W^T[k, j]
            # (bias is added on the host; bf16 hi/lo split, 3 terms)
            psums = [accp.tile([N, JT], FP32, tag="acc", name=f"acc{j}")
                     for j in range(JC // JT)]
            for kc in range(KC):
                rhs = rhsp.tile([P, 2, JC], BF16)
                nc.sync.dma_start(rhs[:], wt[kc * P:(kc + 1) * P, :, :])
                last = kc == KC - 1
                # term-major order reuses the PE stationary across jc tiles
                for term, (xi, wi) in enumerate(((0, 0), (1, 0), (0, 1))):
                    for jc in range(JC // JT):
                        jsl = slice(jc * JT, (jc + 1) * JT)
                        nc.tensor.matmul(psums[jc][:], xb_sb[:, kc, xi, :],
                                         rhs[:, wi, jsl],
                                         start=(kc == 0 and term == 0),
                                         stop=(last and term == 2))
            for jc in range(JC // JT):
                nc.vector.tensor_copy(out_sb[:, jc * JT:(jc + 1) * JT], psums[jc][:])
            nc.sync.dma_start(qkv[:], out_sb[:])

            # Granger: col[i] = x[i,:].wn + gb ; row[j] = hist[j,:].wh
            col_ps = gpsp.tile([N, 1], FP32, tag="col")
            for kc in range(KC):
                nc.tensor.matmul(col_ps[:], xT_sb[:, kc, :], wn_sb[:, kc, :],
                                 start=(kc == 0), stop=False)
            nc.tensor.matmul(col_ps[:], ones_f32[:], gb_sb[:], start=False, stop=True)
            row_ps = gpsp.tile([1, N], FP32, tag="row")
            for kc in range(KC):
                nc.tensor.matmul(row_ps[:], wh_sb[:, kc, :], histT_sb[:, kc, :],
                                 start=(kc == 0), stop=(kc == KC - 1))
            col_sb = const.tile([N, 1], FP32)
            nc.vector.tensor_copy(col_sb[:], col_ps[:])
            row_sb = const.tile([1, N], FP32)
            nc.vector.tensor_copy(row_sb[:], row_ps[:])
            rowb_ps = gpsp.tile([N, N], FP32, tag="rowb")
            nc.tensor.matmul(rowb_ps[:], ones_f32[:], row_sb[:], start=True, stop=True)
            adj_sb = const.tile([N, N], FP32)
            nc.scalar.activation(adj_sb[:], rowb_ps[:],
                                 mybir.ActivationFunctionType.Sigmoid,
                                 bias=col_sb[:])
            nc.vector.tensor_mul(adj_sb[:], adj_sb[:], mask_sb[:])
            nc.sync.dma_start(adj[:], adj_sb[:])
    nc.compile()
    return nc


def _build_launch2():
    nc = bacc.Bacc("TRN2", target_bir_lowering=False, debug=False,
                   num_devices=NCORES)
    NDC = HD // P  # 10 head-dim chunks
    wt2 = nc.dram_tensor("wt2", [FS, 2, E], BF16, kind="ExternalInput")
    qTs = nc.dram_tensor("qTs", [P, NDC, N], FP32, kind="ExternalInput")
    kT = nc.dram_tensor("kT", [P, NDC, N], FP32, kind="ExternalInput")
    v = nc.dram_tensor("v", [N, FS], FP32, kind="ExternalInput")
    outp = nc.dram_tensor("outp", [N, E], FP32, kind="ExternalOutput")

    ET = 512  # out free-dim tile
    FB = FS // P  # 5 f blocks
    with tile.TileContext(nc) as tc:
        with (
            tc.tile_pool(name="const", bufs=1) as const,
            tc.tile_pool(name="w2", bufs=1) as w2p,
            tc.tile_pool(name="att_ps", bufs=1, space="PSUM") as attps,
            tc.tile_pool(name="ot_ps", bufs=2, space="PSUM") as otps,
            tc.tile_pool(name="out_ps", bufs=4, space="PSUM") as outps,
        ):
            # Small attention inputs first (HWDGE FIFO), then the weight
            # slice in e-quarters fc-inner so the out loop can start early.
            qTs_sb = const.tile([P, NDC, N], FP32)
            nc.sync.dma_start(qTs_sb[:], qTs[:])
            kT_sb = const.tile([P, NDC, N], FP32)
            nc.sync.dma_start(kT_sb[:], kT[:])
            v_sb = const.tile([N, FS], FP32)
            nc.sync.dma_start(v_sb[:], v[:])

            w2_sb = []
            for fc in range(FB):
                t = w2p.tile([P, 2, E], BF16, tag=f"w2_{fc}", name=f"w2sb{fc}")
                w2_sb.append(t)
            for eq in range(E // ET):
                for fc in range(FB):
                    nc.sync.dma_start(
                        w2_sb[fc][:, :, eq * ET:(eq + 1) * ET],
                        wt2[fc * P:(fc + 1) * P, :, eq * ET:(eq + 1) * ET])

            ident = const.tile([N, N], FP32)
            make_identity(nc, ident[:])

            # scores[q, t] = sum_d qTs[d,q] kT[d,t]  (q pre-scaled by 1/sqrt(hd))
            sc_ps = attps.tile([N, N], FP32, tag="sc")
            for dc in range(NDC):
                nc.tensor.matmul(sc_ps[:], qTs_sb[:, dc, :], kT_sb[:, dc, :],
                                 start=(dc == 0), stop=(dc == NDC - 1))
            sc_sb = const.tile([N, N], FP32)
            nc.vector.tensor_copy(sc_sb[:], sc_ps[:])
            nmax = const.tile([N, 1], FP32)
            nc.vector.reduce_max(nmax[:], sc_sb[:], axis=mybir.AxisListType.X)
            nc.scalar.mul(nmax[:], nmax[:], -1.0)
            exp_sb = const.tile([N, N], FP32)
            nc.scalar.activation(exp_sb[:], sc_sb[:],
                                 mybir.ActivationFunctionType.Exp, bias=nmax[:])
            ssum = const.tile([N, 1], FP32)
            nc.vector.reduce_sum(ssum[:], exp_sb[:], axis=mybir.AxisListType.X)
            rec = const.tile([N, 1], FP32)
            nc.vector.reciprocal(rec[:], ssum[:])
            attn_sb = const.tile([N, N], FP32)
            nc.vector.tensor_scalar_mul(attn_sb[:], exp_sb[:], rec[:])

            # attn^T via PE transpose, then o^T[d, q] = sum_t v[t,d] attn^T[t,q]
            at_ps = attps.tile([N, N], FP32, tag="at")
            nc.tensor.transpose(at_ps[:], attn_sb[:], ident[:])
            attnT_sb = const.tile([N, N], FP32)
            nc.vector.tensor_copy(attnT_sb[:], at_ps[:])

            # o^T per 128-block, then split into bf16 hi/lo for the big matmul
            oT_sb = const.tile([P, FB, N], FP32)
            ohl_sb = const.tile([P, FB, 2, N], BF16)
            oh32 = const.tile([P, FB, N], FP32)
            for b in range(FB):
                o_ps = otps.tile([P, N], FP32, tag="ot")
                nc.tensor.matmul(o_ps[:], v_sb[:, b * P:(b + 1) * P], attnT_sb[:],
                                 start=True, stop=True)
                nc.vector.tensor_copy(oT_sb[:, b, :], o_ps[:])
                nc.vector.tensor_copy(ohl_sb[:, b, 0, :], o_ps[:])
                nc.vector.tensor_copy(oh32[:, b, :], ohl_sb[:, b, 0, :])
                nc.vector.tensor_sub(oT_sb[:, b, :], oT_sb[:, b, :], oh32[:, b, :])
                nc.vector.tensor_copy(ohl_sb[:, b, 1, :], oT_sb[:, b, :])

            # outp[n, e] = sum_f oT[f,n] * WoutT[f,e]  (partial over f slice)
            out_sb = const.tile([N, E], FP32)
            for ec in range(E // ET):
                esl = slice(ec * ET, (ec + 1) * ET)
                op_ps = outps.tile([N, ET], FP32, tag="out")
                for fc in range(FB):
                    last = fc == FB - 1
                    nc.tensor.matmul(op_ps[:], ohl_sb[:, fc, 0, :],
                                     w2_sb[fc][:, 0, esl],
                                     start=(fc == 0), stop=False)
                    nc.tensor.matmul(op_ps[:], ohl_sb[:, fc, 1, :],
                                     w2_sb[fc][:, 0, esl],
                                     start=False, stop=False)
                    nc.tensor.matmul(op_ps[:], ohl_sb[:, fc, 0, :],
                                     w2_sb[fc][:, 1, esl],
                                     start=False, stop=last)
                nc.vector.tensor_copy(out_sb[:, esl], op_ps[:])
            nc.sync.dma_start(outp[:], out_sb[:])
    nc.compile()
    return nc


def _chunked_T(a2d):
    """[R, C] -> transposed chunk layout [128, R//128, C] (k on partitions)."""
    r, c = a2d.shape
    return np.ascontiguousarray(
        a2d.reshape(r // P, P, c).transpose(1, 0, 2)).astype(np.float32)


def _split_hl(a, axis):
    """Stack bf16 high/low parts of a float32 array along a new axis."""
    ah = a.astype(NPBF)
    al = (a - ah.astype(np.float32)).astype(NPBF)
    return np.ascontiguousarray(np.stack([ah, al], axis=axis))


def kernel(node_states, hist_prev, granger_w, granger_b,
           in_proj_w, in_proj_b, out_proj_w, out_proj_b, light_hertz):
    global LAST_RESULTS
    LAST_RESULTS = []
    node_states = np.asarray(node_states, dtype=np.float32)
    hist_prev = np.asarray(hist_prev, dtype=np.float32)
    granger_w = np.asarray(granger_w, dtype=np.float32)
    granger_b = np.asarray(granger_b, dtype=np.float32)
    in_proj_w = np.asarray(in_proj_w, dtype=np.float32)
    in_proj_b = np.asarray(in_proj_b, dtype=np.float32)
    out_proj_w = np.asarray(out_proj_w, dtype=np.float32)
    out_proj_b = np.asarray(out_proj_b, dtype=np.float32)

    x = node_states.reshape(N, E)
    hist = hist_prev.reshape(N, E)

    xT = _chunked_T(x.T.reshape(E, N))          # [128, 40, 27]
    xb = _split_hl(xT, axis=2)                  # [128, 40, 2, 27] bf16
    histT = _chunked_T(hist.T.reshape(E, N))
    wh = _chunked_T(granger_w[0, :E].reshape(E, 1))
    wn = _chunked_T(granger_w[0, E:].reshape(E, 1))
    gb = granger_b.reshape(1, 1)
    mask = (1.0 - np.eye(N, dtype=np.float32))

    if "l1" not in _NC_CACHE:
        _NC_CACHE["l1"] = _build_launch1()
    if "l2" not in _NC_CACHE:
        _NC_CACHE["l2"] = _build_launch2()

    in_maps1 = []
    for c in range(NCORES):
        sl = slice(c * JC, (c + 1) * JC)
        in_maps1.append({
            "wt": _split_hl(in_proj_w[sl, :].T, axis=1),  # [E, 2, JC] bf16
            "xT": xT, "xb": xb, "histT": histT, "wn": wn, "wh": wh, "gb": gb,
            "mask": mask,
        })
    res1 = run_bass_kernel_spmd(_NC_CACHE["l1"], in_maps1, list(range(NCORES)))
    LAST_RESULTS.append(res1)

    qkv = np.concatenate([res1.results[c]["qkv"] for c in range(NCORES)], axis=1)
    qkv += in_proj_b[None, :]
    causal_adj = np.asarray(res1.results[0]["adj"])

    q = qkv[:, :E]
    k = qkv[:, E:2 * E]
    v = qkv[:, 2 * E:]
    hw = float(np.clip(np.float32(light_hertz) / np.float32(1000.0), 0.1, 1.0))
    qscale = 1.0 / np.sqrt(np.float32(HD))

    in_maps2 = []
    for c in range(NCORES):
        h, half = divmod(c, 2)
        hsl = slice(h * HD, (h + 1) * HD)
        fsl = slice(h * HD + half * FS, h * HD + half * FS + FS)
        in_maps2.append({
            "wt2": _split_hl(out_proj_w[:, fsl].T, axis=1),  # [FS, 2, E] bf16
            "qTs": _chunked_T(np.ascontiguousarray(q[:, hsl].T) * qscale),
            "kT": _chunked_T(np.ascontiguousarray(k[:, hsl].T)),
            "v": np.ascontiguousarray(v[:, fsl]) * np.float32(hw),
        })
    res2 = run_bass_kernel_spmd(_NC_CACHE["l2"], in_maps2, list(range(NCORES)))
    LAST_RESULTS.append(res2)

    out = np.zeros((N, E), dtype=np.float32)
    for c in range(NCORES):
        out += res2.results[c]["outp"]
    out += np.float32(hw) * out_proj_b
    return out.reshape(N, D, 5), causal_adj
